# revision 16
# baseline (speedup 1.0000x reference)
"""Trainium2 Bass kernel for nn_CartesianEquivariantBlock (gnn_message_passing).

Data-parallel over nodes: 30000 nodes sharded 8 ways (3750/core). All
channel-mix (CxC) and path-weight (PxC) params are tiny and replicated.

Per-core device layout: partitions = (h, c) with h = node-half (2) and
c = channel (64); free dim = node index within half (1875), processed in
column tiles of FD. Channel mixing runs on TensorE (bf16, block-diagonal
128x128 stationary built on host). Bilinear spatial contractions run as
fused multi-plane tensor_tensor products (broadcast APs) on VectorE /
GpSimd with weight+accumulate via scalar_tensor_tensor / broadcast
multiplies; ScalarE does pw-scaled PSUM->SBUF copies. Output is a single
(2,64,13,1875) tensor per core, reassembled on host.
"""
import sys

import numpy as np

sys.path.insert(0, "/opt/trn_rl_repo")

import concourse.bass as bass  # noqa: E402
import ml_dtypes  # noqa: E402
import concourse.mybir as mybir  # noqa: E402
from concourse.tile import TileContext  # noqa: E402
from concourse.bass_utils import run_bass_kernel_spmd  # noqa: E402

N_CORES = 8
N_NODES = 30000
NPC = N_NODES // N_CORES      # 3750 nodes per core
C = 64
HALF = NPC // 2               # 1875 columns per node-half
FD = 256                      # node columns per tile (per half)
F32 = mybir.dt.float32
BF16 = mybir.dt.bfloat16
MULT = mybir.AluOpType.mult
ADD = mybir.AluOpType.add
BYPASS = mybir.AluOpType.bypass
AXX = mybir.AxisListType.X

# Mix order in the packed weight tensor (host side prep below):
# rank0: S0, S00a, S00b, S01, S02 -> a0-mixes (w 0..4)
# rank1: V1, V01, V11a, V11b, V12 -> a1-mixes (w 5..9)
# rank2: M2, M02, M12, A22, B22   -> a2-mixes (w 10..14)
MIX_KEYS = [
    ("mix_0", 0), ("mix_00", 0), ("mix_00", 1), ("mix_01", 0), ("mix_02", 0),
    ("mix_1", 0), ("mix_01", 1), ("mix_11", 0), ("mix_11", 1), ("mix_12", 0),
    ("mix_2", 0), ("mix_02", 1), ("mix_12", 1), ("mix_22", 0), ("mix_22", 1),
]
# pw vector columns: b0 paths 0..7 -> 0..7, b1 0..4 -> 8..12, b2 0..8 -> 13..21
PW0, PW1, PW2 = 0, 8, 13
N_PW = 22


def _plane_stride(fd):
    for s in (64, 128, 256, 512):
        if fd <= s:
            return s
    raise ValueError(fd)


def _split_waits(nc, cap=1):
    """walrus ISA structs accept very few sync waits per instruction; move
    excess waits onto same-engine no-ops inserted just before (engine
    streams are in-order, so waiting earlier is equivalent)."""
    cnt = [0]

    def process(block):
        il = getattr(block, "instructions", None)
        if il is not None:
            i = 0
            while i < len(il):
                ins = il[i]
                si = ins.sync_info
                waits = list(si.on_wait) if (si and si.on_wait) else []
                if len(waits) > cap:
                    keep = waits[-cap:]
                    extra = waits[:-cap]
                    pos = i
                    for j in range(0, len(extra), cap):
                        chunk = extra[j:j + cap]
                        cnt[0] += 1
                        nop = mybir.InstNoOp(name="waitnop%d" % cnt[0],
                                             ins=[], outs=[])
                        nop.engine = ins.engine
                        nop.sync_info = mybir.SyncInfo(on_wait=chunk,
                                                       on_update=[])
                        il.insert(pos, nop)
                        pos += 1
                        i += 1
                    ins.sync_info = mybir.SyncInfo(
                        on_wait=keep,
                        on_update=list(si.on_update) if si.on_update else [])
                i += 1
        for sb in getattr(block, "blocks", []) or []:
            process(sb)

    for b in nc.m.functions[0].blocks:
        process(b)


def build_nc():
    nc = bass.Bass()
    a0d = nc.dram_tensor("a0d", [2, C, 1, HALF], BF16, kind="ExternalInput")
    a1d = nc.dram_tensor("a1d", [2, C, 3, HALF], BF16, kind="ExternalInput")
    a2d = nc.dram_tensor("a2d", [2, C, 9, HALF], BF16, kind="ExternalInput")
    wtsd = nc.dram_tensor("wts", [128, 15, 128], BF16, kind="ExternalInput")
    pwvd = nc.dram_tensor("pwv", [128, N_PW], F32, kind="ExternalInput")
    outd = nc.dram_tensor("outd", [2, C, 13, HALF], F32, kind="ExternalOutput")

    with TileContext(nc) as tc:
        with (
            tc.tile_pool(name="const", bufs=1) as cpool,
            tc.tile_pool(name="ain", bufs=2) as apool,
            tc.tile_pool(name="acc", bufs=2) as accpool,
            tc.tile_pool(name="osm", bufs=2) as smpool,
            tc.tile_pool(name="oab", bufs=1) as abpool,
            tc.tile_pool(name="tmp", bufs=6) as tpool,
            tc.tile_pool(name="t9", bufs=6) as t9pool,
            tc.tile_pool(name="pmix", bufs=1, space="PSUM") as ppool,
            tc.tile_pool(name="pstage", bufs=2, space="PSUM") as spool,
        ):
            w_sb = cpool.tile([128, 15, 128], BF16)
            nc.sync.dma_start(w_sb[:], wtsd[:])
            pw_sb = cpool.tile([128, N_PW], F32)
            nc.sync.dma_start(pw_sb[:], pwvd[:])

            def pv(k):
                return pw_sb[:, k:k + 1]

            def W(k):
                return w_sb[:, k, :]

            n_tiles = (HALF + FD - 1) // FD
            for t in range(n_tiles):
                z0 = t * FD
                fd = min(FD, HALF - z0)
                st2 = _plane_stride(fd)
                zsl = slice(z0, z0 + fd)

                a0t = apool.tile([128, 1, FD], BF16, tag="a0t")
                a1t = apool.tile([128, 3, FD], BF16, tag="a1t")
                a2t = apool.tile([128, 9, FD], BF16, tag="a2t")
                nc.sync.dma_start(a0t[:, :, :fd], a0d[:, :, :, zsl])
                nc.sync.dma_start(a1t[:, :, :fd], a1d[:, :, :, zsl])
                nc.sync.dma_start(a2t[:, :, :fd], a2d[:, :, :, zsl])

                outV = accpool.tile([128, 13, FD], F32, tag="outV")

                oV0 = outV[:, 0, :fd]
                oV1 = outV[:, 1:4, :fd]
                oV2 = outV[:, 4:13, :fd]

                def mm(psum_ap, w_idx, rhs_ap):
                    nc.tensor.matmul(psum_ap, W(w_idx), rhs_ap,
                                     start=True, stop=True)

                def vSTT(acc, x, k):
                    # acc += pv[k] * x
                    nc.vector.scalar_tensor_tensor(acc, x, pv(k), acc,
                                                   MULT, ADD)

                def tmp():
                    return tpool.tile([128, FD], F32, tag="tmp",
                                      name="tmp")[:, :fd]

                def tmp9():
                    return t9pool.tile([128, 9, FD], BF16, tag="tmp9",
                                       name="tmp9")

                def tmp9f():
                    return t9pool.tile([128, 9 * FD], F32, tag="tmp9",
                                       name="tmp9f")

                def bc2(ap2, n):
                    # [128, z] -> [128, n, z] broadcast
                    return ap2.rearrange("p (o z) -> p o z", o=1).broadcast_to(
                        (128, n, ap2.shape[-1]))

                def bc_pv3(k, n, z):
                    return pv(k).rearrange("p (a b) -> p a b", a=1).broadcast_to(
                        (128, n, z))

                def bcA(ap3):
                    # [128, a, z] -> [128, a, 3, z]  (broadcast new mid dim)
                    s = ap3.shape
                    return ap3.rearrange("p a (o z) -> p a o z", o=1).broadcast_to(
                        (128, s[1], 3, s[2]))

                def bcB(ap3):
                    # [128, b, z] -> [128, 3, b, z]
                    s = ap3.shape
                    return ap3.rearrange("p b (o z) -> p o b z", o=1).broadcast_to(
                        (128, 3, s[1], s[2]))

                def nat9(t9t):
                    # [128, 9, FD] natural (x, y, z) 4D view
                    return t9t.rearrange("p (d e) z -> p d e z", d=3)[:, :, :, :fd]

                def diag_red(ap3v, out2):
                    # reduce planes {0,4,8} of [128, 9, z] view -> [128, z]
                    dv = ap3v[:, 0:9:4, :].rearrange("p d z -> p z d")
                    nc.vector.tensor_reduce(out2, dv, axis=AXX, op=ADD)

                # ============ R0: rank-0 mixes ============
                ps = ppool.tile([128, 3072], F32, tag="pmix")

                def p0(m):
                    return ps[:, m * 512:m * 512 + fd]

                def p0v(sl):
                    return ps.rearrange("p (a b) -> p a b", b=512)[:, sl, :fd]

                for m in range(5):
                    mm(p0(m), m, a0t[:, 0, :fd])
                nc.scalar.mul(oV0, p0(0), pv(PW0 + 0))
                sb_s = smpool.tile([128, 3, FD], F32, tag="sb_s")
                nc.scalar.copy(sb_s[:, :, :fd], p0v(slice(2, 5)))
                t1 = tmp()
                nc.vector.tensor_tensor(t1, p0(1), sb_s[:, 0, :fd], op=MULT)
                vSTT(oV0, t1, PW0 + 2)

                # ============ R1a: V1, V01 ============
                ps = ppool.tile([128, 3072], F32, tag="pmix")
                for d in range(3):
                    mm(p0(d), 5, a1t[:, d, :fd])
                    mm(p0(3 + d), 6, a1t[:, d, :fd])
                nc.scalar.mul(oV1, p0v(slice(0, 3)), pv(PW1 + 0))
                t3 = tpool.tile([128, 3, FD], F32, tag="t3", name="t3")
                nc.vector.tensor_tensor(t3[:, :, :fd], p0v(slice(3, 6)),
                                        bc2(sb_s[:, 1, :fd], 3), op=MULT)
                vSTT(oV1, t3[:, :, :fd], PW1 + 1)

                # ============ R2a: M2 ============
                ps = ppool.tile([128, 3072], F32, tag="pmix")
                ps3 = ps.rearrange("p (a b) -> p a b", b=st2)

                def p2(dd):
                    return ps[:, dd * st2:dd * st2 + fd]

                for dd in range(9):
                    mm(p2(dd), 10, a2t[:, dd, :fd])
                nc.scalar.mul(oV2, ps3[:, 0:9, :fd], pv(PW2 + 0))
                ttr = tmp()
                diag_red(ps3[:, 0:9, :fd], ttr)
                vSTT(oV0, ttr, PW0 + 1)

                # ============ R1b: V11a, V11b ============
                ps = ppool.tile([128, 3072], F32, tag="pmix")
                for d in range(3):
                    mm(p0(d), 7, a1t[:, d, :fd])
                    mm(p0(3 + d), 8, a1t[:, d, :fd])
                sb_v11b = smpool.tile([128, 3, FD], F32, tag="sb_v11b")
                nc.scalar.copy(sb_v11b[:, :, :fd], p0v(slice(3, 6)))
                t9a = tmp9()
                nc.vector.tensor_tensor(
                    nat9(t9a), bcA(p0v(slice(0, 3))),
                    bcB(sb_v11b[:, :, :fd]), op=MULT)
                vSTT(oV2, t9a[:, :, :fd], PW2 + 2)
                tdot = tmp()
                diag_red(t9a[:, :, :fd], tdot)
                vSTT(oV0, tdot, PW0 + 4)

                # ============ R1c: V12 ============
                ps = ppool.tile([128, 3072], F32, tag="pmix")
                for d in range(3):
                    mm(p0(d), 9, a1t[:, d, :fd])
                sb_v12 = smpool.tile([128, 3, FD], F32, tag="sb_v12")
                nc.scalar.copy(sb_v12[:, :, :fd], p0v(slice(0, 3)))

                # ============ R2b: M02 ============
                ps = ppool.tile([128, 3072], F32, tag="pmix")
                ps3 = ps.rearrange("p (a b) -> p a b", b=st2)
                for dd in range(9):
                    mm(p2(dd), 11, a2t[:, dd, :fd])
                t9b = tmp9()
                nc.vector.tensor_tensor(t9b[:, :, :fd], ps3[:, 0:9, :fd],
                                        bc2(sb_s[:, 2, :fd], 9), op=MULT)
                vSTT(oV2, t9b[:, :, :fd], PW2 + 1)
                t03 = tmp()
                diag_red(t9b[:, :, :fd], t03)
                vSTT(oV0, t03, PW0 + 3)

                # ============ R3: M12 ============
                ps = ppool.tile([128, 3072], F32, tag="pmix")
                ps3 = ps.rearrange("p (a b) -> p a b", b=st2)
                for dd in range(9):
                    mm(p2(dd), 12, a2t[:, dd, :fd])
                # b1_2: out1[f] += pv * sum_d V12_d * M12[3d+f]
                t9L = tmp9f()
                outL = t9L.rearrange("p (f z d) -> p d f z",
                                     f=3, d=3)[:, :, :, :fd]
                in0L = bcA(sb_v12[:, :, :fd])                      # (d, f*, z)
                in1L = ps3[:, 0:9, :st2].rearrange(
                    "p (d f) z -> p d f z", d=3)[:, :, :, :fd]
                nc.vector.tensor_tensor(outL, in0L, in1L, op=MULT)
                t3r = tpool.tile([128, 3, FD], F32, tag="t3", name="t3r")
                nc.vector.tensor_reduce(
                    t3r[:, :, :fd],
                    t9L.rearrange("p (f z d) -> p f z d",
                                  f=3, d=3)[:, :, :fd, :],
                    axis=AXX, op=ADD)
                vSTT(oV1, t3r[:, :, :fd], PW1 + 2)
                # b1_3: out1[f] += pv * sum_d V12_d * M12[3f+d]
                t9M = tmp9f()
                outM = t9M.rearrange("p (f z d) -> p d f z",
                                     f=3, d=3)[:, :, :, :fd]
                in1M = ps3[:, 0:9, :st2].rearrange(
                    "p (f d) z -> p d f z", f=3)[:, :, :, :fd]
                nc.vector.tensor_tensor(outM, in0L, in1M, op=MULT)
                t3s = tpool.tile([128, 3, FD], F32, tag="t3", name="t3s")
                nc.vector.tensor_reduce(
                    t3s[:, :, :fd],
                    t9M.rearrange("p (f z d) -> p f z d",
                                  f=3, d=3)[:, :, :fd, :],
                    axis=AXX, op=ADD)
                vSTT(oV1, t3s[:, :, :fd], PW1 + 3)
                # b1_4: out1_d += pv * V12_d * tr(M12)
                ttr12 = tmp()
                diag_red(ps3[:, 0:9, :fd], ttr12)
                t3t = tpool.tile([128, 3, FD], F32, tag="t3", name="t3t")
                nc.vector.tensor_tensor(t3t[:, :, :fd], sb_v12[:, :, :fd],
                                        bc2(ttr12, 3), op=MULT)
                vSTT(oV1, t3t[:, :, :fd], PW1 + 4)

                # ============ R4: A22 (psum) + B22 (sbuf) ============
                ps = ppool.tile([128, 3072], F32, tag="pmix")
                ps3 = ps.rearrange("p (a b) -> p a b", b=st2)
                for dd in range(9):
                    mm(p2(dd), 13, a2t[:, dd, :fd])
                sb_b = abpool.tile([128, 9, FD], BF16, tag="sb_b")
                sb_a = abpool.tile([128, 9, FD], BF16, tag="sb_a")
                for dd0 in range(0, 9, 2):
                    ndd = min(2, 9 - dd0)
                    stg = spool.tile([128, 512], F32, tag="stage", name="stg")
                    for i in range(ndd):
                        mm(stg[:, i * st2:i * st2 + fd], 14,
                           a2t[:, dd0 + i, :fd])
                    nc.scalar.copy(
                        sb_b[:, dd0:dd0 + ndd, :fd],
                        stg.rearrange("p (a b) -> p a b",
                                      b=st2)[:, 0:ndd, :fd])
                nc.scalar.copy(sb_a[:, 0:5, :fd], ps3[:, 0:5, :fd])
                nc.scalar.copy(sb_a[:, 5:9, :fd], ps3[:, 5:9, :fd])

                ta = tmp()
                diag_red(ps3[:, 0:9, :fd], ta)
                tb = tmp()
                diag_red(sb_b[:, :, :fd], tb)
                # b0_5: tr(A)*tr(B)
                t5 = tmp()
                nc.gpsimd.tensor_tensor(t5, ta, tb, op=MULT)
                vSTT(oV0, t5, PW0 + 5)
                # b0_6: sum(A.B)
                t9c = tmp9f()
                c_out = t9c.rearrange("p (z dd) -> p dd z", dd=9)[:, :, :fd]
                nc.gpsimd.tensor_tensor(c_out, sb_a[:, :, :fd],
                                        sb_b[:, :, :fd], op=MULT)
                r06 = tmp()
                nc.vector.tensor_reduce(
                    r06, t9c.rearrange("p (z dd) -> p z dd", dd=9)[:, :fd, :],
                    axis=AXX, op=ADD)
                vSTT(oV0, r06, PW0 + 6)
                # b0_7: sum(A.B^T)
                t9d = tmp9f()
                d_out = t9d.rearrange("p (z d e) -> p d e z",
                                      d=3, e=3)[:, :, :, :fd]
                in0d = sb_a[:, 0:9, :FD].rearrange(
                    "p (d e) z -> p d e z", d=3)[:, :, :, :fd]
                in1d = sb_b.rearrange("p (e d) z -> p d e z",
                                      e=3)[:, :, :, :fd]
                nc.gpsimd.tensor_tensor(d_out, in0d, in1d, op=MULT)
                r07 = tmp()
                nc.vector.tensor_reduce(
                    r07, t9d.rearrange("p (z de) -> p z de",
                                       de=9)[:, :fd, :],
                    axis=AXX, op=ADD)
                vSTT(oV0, r07, PW0 + 7)
                # b2_3: tr(A)*B product on GpSimd, accumulate on V
                t9e = tmp9()
                nc.gpsimd.tensor_tensor(t9e[:, :, :fd], sb_b[:, :, :fd],
                                        bc2(ta, 9), op=MULT)
                vSTT(oV2, t9e[:, :, :fd], PW2 + 3)
                # b2_8: A*tr(B)
                t9f = tmp9()
                nc.vector.tensor_tensor(t9f[:, :, :fd], sb_a[:, :, :fd],
                                        bc2(tb, 9), op=MULT)
                vSTT(oV2, t9f[:, :, :fd], PW2 + 8)
                # matrix products, V side: b2_4 (A^T B), b2_6 (A B)
                g4 = []
                for d in range(3):
                    t9g = tmp9()
                    nc.vector.tensor_tensor(
                        nat9(t9g), bcA(sb_a[:, 3 * d:3 * d + 3, :fd]),
                        bcB(sb_b[:, 3 * d:3 * d + 3, :fd]), op=MULT)
                    g4.append(t9g)
                nc.vector.tensor_tensor(g4[0][:, :, :fd], g4[0][:, :, :fd],
                                        g4[1][:, :, :fd], op=ADD)
                nc.vector.tensor_tensor(g4[0][:, :, :fd], g4[0][:, :, :fd],
                                        g4[2][:, :, :fd], op=ADD)
                vSTT(oV2, g4[0][:, :, :fd], PW2 + 4)
                g6 = []
                for e in range(3):
                    t9h = tmp9()
                    nc.vector.tensor_tensor(
                        nat9(t9h), bcA(sb_a[:, e:9:3, :fd]),
                        bcB(sb_b[:, 3 * e:3 * e + 3, :fd]), op=MULT)
                    g6.append(t9h)
                nc.vector.tensor_tensor(g6[0][:, :, :fd], g6[0][:, :, :fd],
                                        g6[1][:, :, :fd], op=ADD)
                nc.vector.tensor_tensor(g6[0][:, :, :fd], g6[0][:, :, :fd],
                                        g6[2][:, :, :fd], op=ADD)
                vSTT(oV2, g6[0][:, :, :fd], PW2 + 6)
                # matrix products, G side: b2_5 (A^T B^T), b2_7 (A B^T)
                g5 = []
                for d in range(3):
                    t9i = tmp9()
                    nc.gpsimd.tensor_tensor(
                        nat9(t9i), bcA(sb_a[:, 3 * d:3 * d + 3, :fd]),
                        bcB(sb_b[:, d:9:3, :fd]), op=MULT)
                    g5.append(t9i)
                nc.gpsimd.tensor_tensor(g5[0][:, :, :fd], g5[0][:, :, :fd],
                                        g5[1][:, :, :fd], op=ADD)
                nc.gpsimd.tensor_tensor(g5[0][:, :, :fd], g5[0][:, :, :fd],
                                        g5[2][:, :, :fd], op=ADD)
                vSTT(oV2, g5[0][:, :, :fd], PW2 + 5)
                g7 = []
                for e in range(3):
                    t9j = tmp9()
                    nc.gpsimd.tensor_tensor(
                        nat9(t9j), bcA(sb_a[:, e:9:3, :fd]),
                        bcB(sb_b[:, e:9:3, :fd]), op=MULT)
                    g7.append(t9j)
                nc.gpsimd.tensor_tensor(g7[0][:, :, :fd], g7[0][:, :, :fd],
                                        g7[1][:, :, :fd], op=ADD)
                nc.gpsimd.tensor_tensor(g7[0][:, :, :fd], g7[0][:, :, :fd],
                                        g7[2][:, :, :fd], op=ADD)
                vSTT(oV2, g7[0][:, :, :fd], PW2 + 7)

                nc.sync.dma_start(outd[:, :, :, zsl], outV[:, :, :fd])

    return nc


def _prep_inputs(inputs):
    """Host prep: shard over nodes, pack layouts per core."""
    a0 = np.ascontiguousarray(inputs["a0"], dtype=np.float32)
    a1 = np.ascontiguousarray(inputs["a1"], dtype=np.float32)
    a2 = np.ascontiguousarray(inputs["a2"], dtype=np.float32)
    mixes = {k: np.asarray(inputs[k], dtype=np.float32) for k in
             ("mix_0", "mix_1", "mix_2", "mix_00", "mix_01", "mix_02",
              "mix_11", "mix_12", "mix_22")}
    pw = [np.asarray(inputs["pw%d" % i], dtype=np.float32) for i in range(3)]

    wts = np.zeros((128, 15, 128), np.float32)
    for k, (name, row) in enumerate(MIX_KEYS):
        M = mixes[name][row]
        Mt = M.T
        wts[0:64, k, 0:64] = Mt
        wts[64:128, k, 64:128] = Mt

    pwv = np.zeros((128, N_PW), np.float32)
    cols = np.concatenate([pw[0], pw[1], pw[2]], axis=0)
    pwv[0:64, :] = cols.T
    pwv[64:128, :] = cols.T

    in_maps = []
    for core in range(N_CORES):
        s = slice(core * NPC, (core + 1) * NPC)

        def pack(a, ncomp):
            v = a[s].reshape(2, HALF, C, ncomp)
            return np.ascontiguousarray(v.transpose(0, 2, 3, 1))

        in_maps.append({
            "a0d": pack(a0.reshape(N_NODES, C, 1), 1).astype(ml_dtypes.bfloat16),
            "a1d": pack(a1, 3).astype(ml_dtypes.bfloat16),
            "a2d": pack(a2.reshape(N_NODES, C, 9), 9).astype(ml_dtypes.bfloat16),
            "wts": wts.astype(ml_dtypes.bfloat16),
            "pwv": pwv,
        })
    return in_maps


_CACHED_NC = None


def _get_nc():
    global _CACHED_NC
    if _CACHED_NC is None:
        _CACHED_NC = build_nc()
        _split_waits(_CACHED_NC)
    return _CACHED_NC


def run(inputs, trace=False):
    nc = _get_nc()
    in_maps = _prep_inputs(inputs)
    res = run_bass_kernel_spmd(nc, in_maps, core_ids=list(range(N_CORES)),
                               trace=trace)
    outs = []
    for core in range(N_CORES):
        o = res.results[core]["outd"]            # (2, 64, 13, HALF)
        o = o.transpose(0, 3, 1, 2).reshape(NPC, C, 13)
        outs.append(o)
    full = np.concatenate(outs, axis=0)
    out0 = np.ascontiguousarray(full[:, :, 0])
    out1 = np.ascontiguousarray(full[:, :, 1:4])
    out2 = np.ascontiguousarray(full[:, :, 4:13]).reshape(N_NODES, C, 3, 3)
    return (out0, out1, out2), res


def kernel(**inputs):
    outs, _ = run(inputs, trace=False)
    return outs


# revision 17
# speedup vs baseline: 1.1534x; 1.1534x over previous
"""Trainium2 Bass kernel for nn_CartesianEquivariantBlock (gnn_message_passing).

Data-parallel over nodes: 30000 nodes sharded 8 ways (3750/core). All
channel-mix (CxC) and path-weight (PxC) params are tiny and replicated.

Per-core device layout: partitions = (h, c) with h = node-half (2) and
c = channel (64); free dim = node index within half (1875), processed in
column tiles of FD. Channel mixing runs on TensorE (bf16, block-diagonal
128x128 stationary built on host). Bilinear spatial contractions run as
fused multi-plane tensor_tensor products (broadcast APs) on VectorE /
GpSimd with weight+accumulate via scalar_tensor_tensor / broadcast
multiplies; ScalarE does pw-scaled PSUM->SBUF copies. Output is a single
(2,64,13,1875) tensor per core, reassembled on host.
"""
import sys

import numpy as np

sys.path.insert(0, "/opt/trn_rl_repo")

import concourse.bass as bass  # noqa: E402
import ml_dtypes  # noqa: E402
import concourse.mybir as mybir  # noqa: E402
from concourse.tile import TileContext  # noqa: E402
from concourse.bass_utils import run_bass_kernel_spmd  # noqa: E402

N_CORES = 8
N_NODES = 30000
NPC = N_NODES // N_CORES      # 3750 nodes per core
C = 64
HALF = NPC // 2               # 1875 columns per node-half
FD = 256                      # node columns per tile (per half)
F32 = mybir.dt.float32
BF16 = mybir.dt.bfloat16
MULT = mybir.AluOpType.mult
ADD = mybir.AluOpType.add
BYPASS = mybir.AluOpType.bypass
AXX = mybir.AxisListType.X

# Mix order in the packed weight tensor (host side prep below):
# rank0: S0, S00a, S00b, S01, S02 -> a0-mixes (w 0..4)
# rank1: V1, V01, V11a, V11b, V12 -> a1-mixes (w 5..9)
# rank2: M2, M02, M12, A22, B22   -> a2-mixes (w 10..14)
MIX_KEYS = [
    ("mix_0", 0), ("mix_00", 0), ("mix_00", 1), ("mix_01", 0), ("mix_02", 0),
    ("mix_1", 0), ("mix_01", 1), ("mix_11", 0), ("mix_11", 1), ("mix_12", 0),
    ("mix_2", 0), ("mix_02", 1), ("mix_12", 1), ("mix_22", 0), ("mix_22", 1),
]
# pw vector columns: b0 paths 0..7 -> 0..7, b1 0..4 -> 8..12, b2 0..8 -> 13..21
PW0, PW1, PW2 = 0, 8, 13
N_PW = 22


def _plane_stride(fd):
    for s in (64, 128, 256, 512):
        if fd <= s:
            return s
    raise ValueError(fd)


def _split_waits(nc, cap=1):
    """walrus ISA structs accept very few sync waits per instruction; move
    excess waits onto same-engine no-ops inserted just before (engine
    streams are in-order, so waiting earlier is equivalent)."""
    cnt = [0]

    def process(block):
        il = getattr(block, "instructions", None)
        if il is not None:
            i = 0
            while i < len(il):
                ins = il[i]
                si = ins.sync_info
                waits = list(si.on_wait) if (si and si.on_wait) else []
                if len(waits) > cap:
                    keep = waits[-cap:]
                    extra = waits[:-cap]
                    pos = i
                    for j in range(0, len(extra), cap):
                        chunk = extra[j:j + cap]
                        cnt[0] += 1
                        nop = mybir.InstNoOp(name="waitnop%d" % cnt[0],
                                             ins=[], outs=[])
                        nop.engine = ins.engine
                        nop.sync_info = mybir.SyncInfo(on_wait=chunk,
                                                       on_update=[])
                        il.insert(pos, nop)
                        pos += 1
                        i += 1
                    ins.sync_info = mybir.SyncInfo(
                        on_wait=keep,
                        on_update=list(si.on_update) if si.on_update else [])
                i += 1
        for sb in getattr(block, "blocks", []) or []:
            process(sb)

    for b in nc.m.functions[0].blocks:
        process(b)


def build_nc():
    nc = bass.Bass()
    a0d = nc.dram_tensor("a0d", [2, C, 1, HALF], BF16, kind="ExternalInput")
    a1d = nc.dram_tensor("a1d", [2, C, 3, HALF], BF16, kind="ExternalInput")
    a2d = nc.dram_tensor("a2d", [2, C, 9, HALF], BF16, kind="ExternalInput")
    wtsd = nc.dram_tensor("wts", [128, 15, 128], BF16, kind="ExternalInput")
    pwvd = nc.dram_tensor("pwv", [128, N_PW], F32, kind="ExternalInput")
    outd = nc.dram_tensor("outd", [2, C, 13, HALF], F32, kind="ExternalOutput")

    with TileContext(nc) as tc:
        with (
            tc.tile_pool(name="const", bufs=1) as cpool,
            tc.tile_pool(name="ain", bufs=2) as apool,
            tc.tile_pool(name="acc", bufs=2) as accpool,
            tc.tile_pool(name="osm", bufs=2) as smpool,
            tc.tile_pool(name="oab", bufs=1) as abpool,
            tc.tile_pool(name="tmp", bufs=6) as tpool,
            tc.tile_pool(name="t9", bufs=4) as t9pool,
            tc.tile_pool(name="pmix", bufs=1, space="PSUM") as ppool,
            tc.tile_pool(name="pstage", bufs=2, space="PSUM") as spool,
        ):
            w_sb = cpool.tile([128, 15, 128], BF16)
            nc.sync.dma_start(w_sb[:], wtsd[:])
            pw_sb = cpool.tile([128, N_PW], F32)
            nc.sync.dma_start(pw_sb[:], pwvd[:])

            def pv(k):
                return pw_sb[:, k:k + 1]

            def W(k):
                return w_sb[:, k, :]

            n_tiles = (HALF + FD - 1) // FD
            for t in range(n_tiles):
                z0 = t * FD
                fd = min(FD, HALF - z0)
                st2 = _plane_stride(fd)
                zsl = slice(z0, z0 + fd)

                a0t = apool.tile([128, 1, FD], BF16, tag="a0t")
                a1t = apool.tile([128, 3, FD], BF16, tag="a1t")
                a2t = apool.tile([128, 9, FD], BF16, tag="a2t")
                nc.sync.dma_start(a0t[:, :, :fd], a0d[:, :, :, zsl])
                nc.sync.dma_start(a1t[:, :, :fd], a1d[:, :, :, zsl])
                nc.sync.dma_start(a2t[:, :, :fd], a2d[:, :, :, zsl])

                outV = accpool.tile([128, 13, FD], F32, tag="outV")

                oV0 = outV[:, 0, :fd]
                oV1 = outV[:, 1:4, :fd]
                oV2 = outV[:, 4:13, :fd]

                def mm(psum_ap, w_idx, rhs_ap):
                    nc.tensor.matmul(psum_ap, W(w_idx), rhs_ap,
                                     start=True, stop=True)

                def vSTT(acc, x, k):
                    # acc += pv[k] * x
                    nc.vector.scalar_tensor_tensor(acc, x, pv(k), acc,
                                                   MULT, ADD)

                def tmp():
                    return tpool.tile([128, FD], F32, tag="tmp",
                                      name="tmp")[:, :fd]

                def tmp9():
                    return t9pool.tile([128, 9, FD], BF16, tag="tmp9",
                                       name="tmp9")

                def tmp9f():
                    return t9pool.tile([128, 9 * FD], F32, tag="tmp9",
                                       name="tmp9f")

                def bc2(ap2, n):
                    # [128, z] -> [128, n, z] broadcast
                    return ap2.rearrange("p (o z) -> p o z", o=1).broadcast_to(
                        (128, n, ap2.shape[-1]))

                def bc_pv3(k, n, z):
                    return pv(k).rearrange("p (a b) -> p a b", a=1).broadcast_to(
                        (128, n, z))

                def bcA(ap3):
                    # [128, a, z] -> [128, a, 3, z]  (broadcast new mid dim)
                    s = ap3.shape
                    return ap3.rearrange("p a (o z) -> p a o z", o=1).broadcast_to(
                        (128, s[1], 3, s[2]))

                def bcB(ap3):
                    # [128, b, z] -> [128, 3, b, z]
                    s = ap3.shape
                    return ap3.rearrange("p b (o z) -> p o b z", o=1).broadcast_to(
                        (128, 3, s[1], s[2]))

                def nat9(t9t):
                    # [128, 9, FD] natural (x, y, z) 4D view
                    return t9t.rearrange("p (d e) z -> p d e z", d=3)[:, :, :, :fd]

                def diag_red(ap3v, out2):
                    # reduce planes {0,4,8} of [128, 9, z] view -> [128, z]
                    dv = ap3v[:, 0:9:4, :].rearrange("p d z -> p z d")
                    nc.vector.tensor_reduce(out2, dv, axis=AXX, op=ADD)

                # ============ R0: rank-0 mixes ============
                ps = ppool.tile([128, 3072], F32, tag="pmix")

                def p0(m):
                    return ps[:, m * 512:m * 512 + fd]

                def p0v(sl):
                    return ps.rearrange("p (a b) -> p a b", b=512)[:, sl, :fd]

                for m in range(5):
                    mm(p0(m), m, a0t[:, 0, :fd])
                nc.scalar.mul(oV0, p0(0), pv(PW0 + 0))
                sb_s = smpool.tile([128, 3, FD], F32, tag="sb_s")
                nc.scalar.copy(sb_s[:, :, :fd], p0v(slice(2, 5)))
                t1 = tmp()
                nc.vector.tensor_tensor(t1, p0(1), sb_s[:, 0, :fd], op=MULT)
                vSTT(oV0, t1, PW0 + 2)

                # ============ R1a: V1, V01 ============
                ps = ppool.tile([128, 3072], F32, tag="pmix")
                for d in range(3):
                    mm(p0(d), 5, a1t[:, d, :fd])
                    mm(p0(3 + d), 6, a1t[:, d, :fd])
                nc.scalar.mul(oV1, p0v(slice(0, 3)), pv(PW1 + 0))
                t3 = tpool.tile([128, 3, FD], F32, tag="t3", name="t3")
                nc.vector.tensor_tensor(t3[:, :, :fd], p0v(slice(3, 6)),
                                        bc2(sb_s[:, 1, :fd], 3), op=MULT)
                vSTT(oV1, t3[:, :, :fd], PW1 + 1)

                # ============ R2a: M2 ============
                ps = ppool.tile([128, 3072], F32, tag="pmix")
                ps3 = ps.rearrange("p (a b) -> p a b", b=st2)

                def p2(dd):
                    return ps[:, dd * st2:dd * st2 + fd]

                for dd in range(9):
                    mm(p2(dd), 10, a2t[:, dd, :fd])
                nc.scalar.mul(oV2, ps3[:, 0:9, :fd], pv(PW2 + 0))
                ttr = tmp()
                diag_red(ps3[:, 0:9, :fd], ttr)
                vSTT(oV0, ttr, PW0 + 1)

                # ============ R1b: V11a, V11b ============
                ps = ppool.tile([128, 3072], F32, tag="pmix")
                for d in range(3):
                    mm(p0(d), 7, a1t[:, d, :fd])
                    mm(p0(3 + d), 8, a1t[:, d, :fd])
                sb_v11b = smpool.tile([128, 3, FD], F32, tag="sb_v11b")
                nc.scalar.copy(sb_v11b[:, :, :fd], p0v(slice(3, 6)))
                t9a = tmp9()
                nc.vector.tensor_tensor(
                    nat9(t9a), bcA(p0v(slice(0, 3))),
                    bcB(sb_v11b[:, :, :fd]), op=MULT)
                vSTT(oV2, t9a[:, :, :fd], PW2 + 2)
                tdot = tmp()
                diag_red(t9a[:, :, :fd], tdot)
                vSTT(oV0, tdot, PW0 + 4)

                # ============ R1c: V12 ============
                ps = ppool.tile([128, 3072], F32, tag="pmix")
                for d in range(3):
                    mm(p0(d), 9, a1t[:, d, :fd])
                sb_v12 = smpool.tile([128, 3, FD], F32, tag="sb_v12")
                nc.scalar.copy(sb_v12[:, :, :fd], p0v(slice(0, 3)))

                # ============ R2b: M02 ============
                ps = ppool.tile([128, 3072], F32, tag="pmix")
                ps3 = ps.rearrange("p (a b) -> p a b", b=st2)
                for dd in range(9):
                    mm(p2(dd), 11, a2t[:, dd, :fd])
                t9b = tmp9()
                nc.vector.tensor_tensor(t9b[:, :, :fd], ps3[:, 0:9, :fd],
                                        bc2(sb_s[:, 2, :fd], 9), op=MULT)
                vSTT(oV2, t9b[:, :, :fd], PW2 + 1)
                t03 = tmp()
                diag_red(t9b[:, :, :fd], t03)
                vSTT(oV0, t03, PW0 + 3)

                # ============ R3: M12 ============
                ps = ppool.tile([128, 3072], F32, tag="pmix")
                ps3 = ps.rearrange("p (a b) -> p a b", b=st2)
                for dd in range(9):
                    mm(p2(dd), 12, a2t[:, dd, :fd])
                # b1_2: out1[f] += pv * sum_d V12_d * M12[3d+f]
                t9L = tmp9f()
                outL = t9L.rearrange("p (f z d) -> p d f z",
                                     f=3, d=3)[:, :, :, :fd]
                in0L = bcA(sb_v12[:, :, :fd])                      # (d, f*, z)
                in1L = ps3[:, 0:9, :st2].rearrange(
                    "p (d f) z -> p d f z", d=3)[:, :, :, :fd]
                nc.vector.tensor_tensor(outL, in0L, in1L, op=MULT)
                t3r = tpool.tile([128, 3, FD], F32, tag="t3", name="t3r")
                nc.vector.tensor_reduce(
                    t3r[:, :, :fd],
                    t9L.rearrange("p (f z d) -> p f z d",
                                  f=3, d=3)[:, :, :fd, :],
                    axis=AXX, op=ADD)
                vSTT(oV1, t3r[:, :, :fd], PW1 + 2)
                # b1_3: out1[f] += pv * sum_d V12_d * M12[3f+d]
                t9M = tmp9f()
                outM = t9M.rearrange("p (f z d) -> p d f z",
                                     f=3, d=3)[:, :, :, :fd]
                in1M = ps3[:, 0:9, :st2].rearrange(
                    "p (f d) z -> p d f z", f=3)[:, :, :, :fd]
                nc.vector.tensor_tensor(outM, in0L, in1M, op=MULT)
                t3s = tpool.tile([128, 3, FD], F32, tag="t3", name="t3s")
                nc.vector.tensor_reduce(
                    t3s[:, :, :fd],
                    t9M.rearrange("p (f z d) -> p f z d",
                                  f=3, d=3)[:, :, :fd, :],
                    axis=AXX, op=ADD)
                vSTT(oV1, t3s[:, :, :fd], PW1 + 3)
                # b1_4: out1_d += pv * V12_d * tr(M12)
                ttr12 = tmp()
                diag_red(ps3[:, 0:9, :fd], ttr12)
                t3t = tpool.tile([128, 3, FD], F32, tag="t3", name="t3t")
                nc.vector.tensor_tensor(t3t[:, :, :fd], sb_v12[:, :, :fd],
                                        bc2(ttr12, 3), op=MULT)
                vSTT(oV1, t3t[:, :, :fd], PW1 + 4)

                # ============ R4: A22 (psum) + B22 (sbuf) ============
                ps = ppool.tile([128, 3072], F32, tag="pmix")
                ps3 = ps.rearrange("p (a b) -> p a b", b=st2)
                for dd in range(9):
                    mm(p2(dd), 13, a2t[:, dd, :fd])
                sb_b = abpool.tile([128, 9, FD], BF16, tag="sb_b")
                sb_a = abpool.tile([128, 9, FD], BF16, tag="sb_a")
                for dd0 in range(0, 9, 2):
                    ndd = min(2, 9 - dd0)
                    stg = spool.tile([128, 512], F32, tag="stage", name="stg")
                    for i in range(ndd):
                        mm(stg[:, i * st2:i * st2 + fd], 14,
                           a2t[:, dd0 + i, :fd])
                    nc.scalar.copy(
                        sb_b[:, dd0:dd0 + ndd, :fd],
                        stg.rearrange("p (a b) -> p a b",
                                      b=st2)[:, 0:ndd, :fd])
                nc.scalar.copy(sb_a[:, 0:5, :fd], ps3[:, 0:5, :fd])
                nc.scalar.copy(sb_a[:, 5:9, :fd], ps3[:, 5:9, :fd])

                ta = tmp()
                diag_red(ps3[:, 0:9, :fd], ta)
                tb = tmp()
                diag_red(sb_b[:, :, :fd], tb)
                # b0_5: tr(A)*tr(B)
                t5 = tmp()
                nc.gpsimd.tensor_tensor(t5, ta, tb, op=MULT)
                vSTT(oV0, t5, PW0 + 5)
                # b0_6: sum(A.B)
                t9c = tmp9f()
                c_out = t9c.rearrange("p (z dd) -> p dd z", dd=9)[:, :, :fd]
                nc.vector.tensor_tensor(c_out, sb_a[:, :, :fd],
                                        sb_b[:, :, :fd], op=MULT)
                r06 = tmp()
                nc.vector.tensor_reduce(
                    r06, t9c.rearrange("p (z dd) -> p z dd", dd=9)[:, :fd, :],
                    axis=AXX, op=ADD)
                vSTT(oV0, r06, PW0 + 6)
                # b0_7: sum(A.B^T)
                t9d = tmp9f()
                d_out = t9d.rearrange("p (z d e) -> p d e z",
                                      d=3, e=3)[:, :, :, :fd]
                in0d = sb_a[:, 0:9, :FD].rearrange(
                    "p (d e) z -> p d e z", d=3)[:, :, :, :fd]
                in1d = sb_b.rearrange("p (e d) z -> p d e z",
                                      e=3)[:, :, :, :fd]
                nc.vector.tensor_tensor(d_out, in0d, in1d, op=MULT)
                r07 = tmp()
                nc.vector.tensor_reduce(
                    r07, t9d.rearrange("p (z de) -> p z de",
                                       de=9)[:, :fd, :],
                    axis=AXX, op=ADD)
                vSTT(oV0, r07, PW0 + 7)
                # b2_3: tr(A)*B product on GpSimd, accumulate on V
                t9e = tmp9()
                nc.gpsimd.tensor_tensor(t9e[:, :, :fd], sb_b[:, :, :fd],
                                        bc2(ta, 9), op=MULT)
                vSTT(oV2, t9e[:, :, :fd], PW2 + 3)
                # b2_8: A*tr(B)
                t9f = tmp9()
                nc.vector.tensor_tensor(t9f[:, :, :fd], sb_a[:, :, :fd],
                                        bc2(tb, 9), op=MULT)
                vSTT(oV2, t9f[:, :, :fd], PW2 + 8)
                # matrix products, V side: b2_4 (A^T B), b2_6 (A B)
                for d in range(3):
                    t9g = tmp9()
                    nc.vector.tensor_tensor(
                        nat9(t9g), bcA(sb_a[:, 3 * d:3 * d + 3, :fd]),
                        bcB(sb_b[:, 3 * d:3 * d + 3, :fd]), op=MULT)
                    vSTT(oV2, t9g[:, :, :fd], PW2 + 4)
                for e in range(3):
                    t9h = tmp9()
                    nc.vector.tensor_tensor(
                        nat9(t9h), bcA(sb_a[:, e:9:3, :fd]),
                        bcB(sb_b[:, 3 * e:3 * e + 3, :fd]), op=MULT)
                    vSTT(oV2, t9h[:, :, :fd], PW2 + 6)
                # matrix products, G side: b2_5 (A^T B^T), b2_7 (A B^T)
                for d in range(3):
                    t9i = tmp9()
                    nc.gpsimd.tensor_tensor(
                        nat9(t9i), bcA(sb_a[:, 3 * d:3 * d + 3, :fd]),
                        bcB(sb_b[:, d:9:3, :fd]), op=MULT)
                    vSTT(oV2, t9i[:, :, :fd], PW2 + 5)
                for e in range(3):
                    t9j = tmp9()
                    nc.gpsimd.tensor_tensor(
                        nat9(t9j), bcA(sb_a[:, e:9:3, :fd]),
                        bcB(sb_b[:, e:9:3, :fd]), op=MULT)
                    vSTT(oV2, t9j[:, :, :fd], PW2 + 7)

                nc.sync.dma_start(outd[:, :, :, zsl], outV[:, :, :fd])

    return nc


def _prep_inputs(inputs):
    """Host prep: shard over nodes, pack layouts per core."""
    a0 = np.ascontiguousarray(inputs["a0"], dtype=np.float32)
    a1 = np.ascontiguousarray(inputs["a1"], dtype=np.float32)
    a2 = np.ascontiguousarray(inputs["a2"], dtype=np.float32)
    mixes = {k: np.asarray(inputs[k], dtype=np.float32) for k in
             ("mix_0", "mix_1", "mix_2", "mix_00", "mix_01", "mix_02",
              "mix_11", "mix_12", "mix_22")}
    pw = [np.asarray(inputs["pw%d" % i], dtype=np.float32) for i in range(3)]

    wts = np.zeros((128, 15, 128), np.float32)
    for k, (name, row) in enumerate(MIX_KEYS):
        M = mixes[name][row]
        Mt = M.T
        wts[0:64, k, 0:64] = Mt
        wts[64:128, k, 64:128] = Mt

    pwv = np.zeros((128, N_PW), np.float32)
    cols = np.concatenate([pw[0], pw[1], pw[2]], axis=0)
    pwv[0:64, :] = cols.T
    pwv[64:128, :] = cols.T

    in_maps = []
    for core in range(N_CORES):
        s = slice(core * NPC, (core + 1) * NPC)

        def pack(a, ncomp):
            v = a[s].reshape(2, HALF, C, ncomp)
            return np.ascontiguousarray(v.transpose(0, 2, 3, 1))

        in_maps.append({
            "a0d": pack(a0.reshape(N_NODES, C, 1), 1).astype(ml_dtypes.bfloat16),
            "a1d": pack(a1, 3).astype(ml_dtypes.bfloat16),
            "a2d": pack(a2.reshape(N_NODES, C, 9), 9).astype(ml_dtypes.bfloat16),
            "wts": wts.astype(ml_dtypes.bfloat16),
            "pwv": pwv,
        })
    return in_maps


_CACHED_NC = None


def _get_nc():
    global _CACHED_NC
    if _CACHED_NC is None:
        _CACHED_NC = build_nc()
        _split_waits(_CACHED_NC)
    return _CACHED_NC


def run(inputs, trace=False):
    nc = _get_nc()
    in_maps = _prep_inputs(inputs)
    res = run_bass_kernel_spmd(nc, in_maps, core_ids=list(range(N_CORES)),
                               trace=trace)
    outs = []
    for core in range(N_CORES):
        o = res.results[core]["outd"]            # (2, 64, 13, HALF)
        o = o.transpose(0, 3, 1, 2).reshape(NPC, C, 13)
        outs.append(o)
    full = np.concatenate(outs, axis=0)
    out0 = np.ascontiguousarray(full[:, :, 0])
    out1 = np.ascontiguousarray(full[:, :, 1:4])
    out2 = np.ascontiguousarray(full[:, :, 4:13]).reshape(N_NODES, C, 3, 3)
    return (out0, out1, out2), res


def kernel(**inputs):
    outs, _ = run(inputs, trace=False)
    return outs


# revision 18
# speedup vs baseline: 1.2281x; 1.0647x over previous
"""Trainium2 Bass kernel for nn_CartesianEquivariantBlock (gnn_message_passing).

Data-parallel over nodes: 30000 nodes sharded 8 ways (3750/core). All
channel-mix (CxC) and path-weight (PxC) params are tiny and replicated.

Per-core device layout: partitions = (h, c) with h = node-half (2) and
c = channel (64); free dim = node index within half (1875), processed in
column tiles of FD. Channel mixing runs on TensorE (bf16, block-diagonal
128x128 stationary built on host). Bilinear spatial contractions run as
fused multi-plane tensor_tensor products (broadcast APs) on VectorE /
GpSimd with weight+accumulate via scalar_tensor_tensor / broadcast
multiplies; ScalarE does pw-scaled PSUM->SBUF copies. Output is a single
(2,64,13,1875) tensor per core, reassembled on host.
"""
import sys

import numpy as np

sys.path.insert(0, "/opt/trn_rl_repo")

import concourse.bass as bass  # noqa: E402
import ml_dtypes  # noqa: E402
import concourse.mybir as mybir  # noqa: E402
from concourse.tile import TileContext  # noqa: E402
from concourse.bass_utils import run_bass_kernel_spmd  # noqa: E402

N_CORES = 8
N_NODES = 30000
NPC = N_NODES // N_CORES      # 3750 nodes per core
C = 64
HALF = NPC // 2               # 1875 columns per node-half
FD = 256                      # node columns per tile (per half)
F32 = mybir.dt.float32
BF16 = mybir.dt.bfloat16
MULT = mybir.AluOpType.mult
ADD = mybir.AluOpType.add
BYPASS = mybir.AluOpType.bypass
AXX = mybir.AxisListType.X

# Mix order in the packed weight tensor (host side prep below):
# rank0: S0, S00a, S00b, S01, S02 -> a0-mixes (w 0..4)
# rank1: V1, V01, V11a, V11b, V12 -> a1-mixes (w 5..9)
# rank2: M2, M02, M12, A22, B22   -> a2-mixes (w 10..14)
MIX_KEYS = [
    ("mix_0", 0), ("mix_00", 0), ("mix_00", 1), ("mix_01", 0), ("mix_02", 0),
    ("mix_1", 0), ("mix_01", 1), ("mix_11", 0), ("mix_11", 1), ("mix_12", 0),
    ("mix_2", 0), ("mix_02", 1), ("mix_12", 1), ("mix_22", 0), ("mix_22", 1),
]
# pw vector columns: b0 paths 0..7 -> 0..7, b1 0..4 -> 8..12, b2 0..8 -> 13..21
PW0, PW1, PW2 = 0, 8, 13
N_PW = 22


def _plane_stride(fd):
    for s in (64, 128, 256, 512):
        if fd <= s:
            return s
    raise ValueError(fd)


def _split_waits(nc, cap=1):
    """walrus ISA structs accept very few sync waits per instruction; move
    excess waits onto same-engine no-ops inserted just before (engine
    streams are in-order, so waiting earlier is equivalent)."""
    cnt = [0]

    def process(block):
        il = getattr(block, "instructions", None)
        if il is not None:
            i = 0
            while i < len(il):
                ins = il[i]
                si = ins.sync_info
                waits = list(si.on_wait) if (si and si.on_wait) else []
                if len(waits) > cap:
                    keep = waits[-cap:]
                    extra = waits[:-cap]
                    pos = i
                    for j in range(0, len(extra), cap):
                        chunk = extra[j:j + cap]
                        cnt[0] += 1
                        nop = mybir.InstNoOp(name="waitnop%d" % cnt[0],
                                             ins=[], outs=[])
                        nop.engine = ins.engine
                        nop.sync_info = mybir.SyncInfo(on_wait=chunk,
                                                       on_update=[])
                        il.insert(pos, nop)
                        pos += 1
                        i += 1
                    ins.sync_info = mybir.SyncInfo(
                        on_wait=keep,
                        on_update=list(si.on_update) if si.on_update else [])
                i += 1
        for sb in getattr(block, "blocks", []) or []:
            process(sb)

    for b in nc.m.functions[0].blocks:
        process(b)


def build_nc():
    nc = bass.Bass()
    a0d = nc.dram_tensor("a0d", [2, C, 1, HALF], BF16, kind="ExternalInput")
    a1d = nc.dram_tensor("a1d", [2, C, 3, HALF], BF16, kind="ExternalInput")
    a2d = nc.dram_tensor("a2d", [2, C, 9, HALF], BF16, kind="ExternalInput")
    wtsd = nc.dram_tensor("wts", [128, 15, 128], BF16, kind="ExternalInput")
    pwvd = nc.dram_tensor("pwv", [128, N_PW], F32, kind="ExternalInput")
    outd = nc.dram_tensor("outd", [2, C, 13, HALF], F32, kind="ExternalOutput")

    with TileContext(nc) as tc:
        with (
            tc.tile_pool(name="const", bufs=1) as cpool,
            tc.tile_pool(name="ain", bufs=2) as apool,
            tc.tile_pool(name="acc", bufs=2) as accpool,
            tc.tile_pool(name="osm", bufs=2) as smpool,
            tc.tile_pool(name="oab", bufs=1) as abpool,
            tc.tile_pool(name="tmp", bufs=6) as tpool,
            tc.tile_pool(name="t9", bufs=6) as t9pool,
            tc.tile_pool(name="pmix", bufs=1, space="PSUM") as ppool,
            tc.tile_pool(name="pstage", bufs=2, space="PSUM") as spool,
        ):
            w_sb = cpool.tile([128, 15, 128], BF16)
            nc.sync.dma_start(w_sb[:], wtsd[:])
            pw_sb = cpool.tile([128, N_PW], F32)
            nc.sync.dma_start(pw_sb[:], pwvd[:])

            def pv(k):
                return pw_sb[:, k:k + 1]

            def W(k):
                return w_sb[:, k, :]

            n_tiles = (HALF + FD - 1) // FD
            for t in range(n_tiles):
                z0 = t * FD
                fd = min(FD, HALF - z0)
                st2 = _plane_stride(fd)
                zsl = slice(z0, z0 + fd)

                a0t = apool.tile([128, 1, FD], BF16, tag="a0t")
                a1t = apool.tile([128, 3, FD], BF16, tag="a1t")
                a2t = apool.tile([128, 9, FD], BF16, tag="a2t")
                nc.sync.dma_start(a0t[:, :, :fd], a0d[:, :, :, zsl])
                nc.sync.dma_start(a1t[:, :, :fd], a1d[:, :, :, zsl])
                nc.sync.dma_start(a2t[:, :, :fd], a2d[:, :, :, zsl])

                outV = accpool.tile([128, 13, FD], F32, tag="outV")

                oV0 = outV[:, 0, :fd]
                oV1 = outV[:, 1:4, :fd]
                oV2 = outV[:, 4:13, :fd]

                def mm(psum_ap, w_idx, rhs_ap):
                    nc.tensor.matmul(psum_ap, W(w_idx), rhs_ap,
                                     start=True, stop=True)

                def vSTT(acc, x, k):
                    # acc += pv[k] * x
                    nc.vector.scalar_tensor_tensor(acc, x, pv(k), acc,
                                                   MULT, ADD)

                def tmp():
                    return tpool.tile([128, FD], F32, tag="tmp",
                                      name="tmp")[:, :fd]

                def tmp9():
                    return t9pool.tile([128, 9, FD], BF16, tag="tmp9",
                                       name="tmp9")

                def tmp9f():
                    return t9pool.tile([128, 9 * FD], F32, tag="tmp9",
                                       name="tmp9f")

                def bc2(ap2, n):
                    # [128, z] -> [128, n, z] broadcast
                    return ap2.rearrange("p (o z) -> p o z", o=1).broadcast_to(
                        (128, n, ap2.shape[-1]))

                def bc_pv3(k, n, z):
                    return pv(k).rearrange("p (a b) -> p a b", a=1).broadcast_to(
                        (128, n, z))

                def bcA(ap3):
                    # [128, a, z] -> [128, a, 3, z]  (broadcast new mid dim)
                    s = ap3.shape
                    return ap3.rearrange("p a (o z) -> p a o z", o=1).broadcast_to(
                        (128, s[1], 3, s[2]))

                def bcB(ap3):
                    # [128, b, z] -> [128, 3, b, z]
                    s = ap3.shape
                    return ap3.rearrange("p b (o z) -> p o b z", o=1).broadcast_to(
                        (128, 3, s[1], s[2]))

                def nat9(t9t):
                    # [128, 9, FD] natural (x, y, z) 4D view
                    return t9t.rearrange("p (d e) z -> p d e z", d=3)[:, :, :, :fd]

                def diag_red(ap3v, out2):
                    # reduce planes {0,4,8} of [128, 9, z] view -> [128, z]
                    dv = ap3v[:, 0:9:4, :].rearrange("p d z -> p z d")
                    nc.vector.tensor_reduce(out2, dv, axis=AXX, op=ADD)

                # ============ R0: rank-0 mixes ============
                ps = ppool.tile([128, 3072], F32, tag="pmix")

                def p0(m):
                    return ps[:, m * 512:m * 512 + fd]

                def p0v(sl):
                    return ps.rearrange("p (a b) -> p a b", b=512)[:, sl, :fd]

                for m in range(5):
                    mm(p0(m), m, a0t[:, 0, :fd])
                nc.scalar.mul(oV0, p0(0), pv(PW0 + 0))
                sb_s = smpool.tile([128, 3, FD], F32, tag="sb_s")
                nc.scalar.copy(sb_s[:, :, :fd], p0v(slice(2, 5)))
                t1 = tmp()
                nc.vector.tensor_tensor(t1, p0(1), sb_s[:, 0, :fd], op=MULT)
                vSTT(oV0, t1, PW0 + 2)

                # ============ R1a: V1, V01 ============
                ps = ppool.tile([128, 3072], F32, tag="pmix")
                for d in range(3):
                    mm(p0(d), 5, a1t[:, d, :fd])
                    mm(p0(3 + d), 6, a1t[:, d, :fd])
                nc.scalar.mul(oV1, p0v(slice(0, 3)), pv(PW1 + 0))
                t3 = tpool.tile([128, 3, FD], F32, tag="t3", name="t3")
                nc.vector.tensor_tensor(t3[:, :, :fd], p0v(slice(3, 6)),
                                        bc2(sb_s[:, 1, :fd], 3), op=MULT)
                vSTT(oV1, t3[:, :, :fd], PW1 + 1)

                # ============ R2a: M2 ============
                ps = ppool.tile([128, 3072], F32, tag="pmix")
                ps3 = ps.rearrange("p (a b) -> p a b", b=st2)

                def p2(dd):
                    return ps[:, dd * st2:dd * st2 + fd]

                for dd in range(9):
                    mm(p2(dd), 10, a2t[:, dd, :fd])
                nc.scalar.mul(oV2, ps3[:, 0:9, :fd], pv(PW2 + 0))
                ttr = tmp()
                diag_red(ps3[:, 0:9, :fd], ttr)
                vSTT(oV0, ttr, PW0 + 1)

                # ============ R1b: V11a, V11b ============
                ps = ppool.tile([128, 3072], F32, tag="pmix")
                for d in range(3):
                    mm(p0(d), 7, a1t[:, d, :fd])
                    mm(p0(3 + d), 8, a1t[:, d, :fd])
                sb_v11b = smpool.tile([128, 3, FD], F32, tag="sb_v11b")
                nc.scalar.copy(sb_v11b[:, :, :fd], p0v(slice(3, 6)))
                t9a = tmp9()
                nc.vector.tensor_tensor(
                    nat9(t9a), bcA(p0v(slice(0, 3))),
                    bcB(sb_v11b[:, :, :fd]), op=MULT)
                vSTT(oV2, t9a[:, :, :fd], PW2 + 2)
                tdot = tmp()
                diag_red(t9a[:, :, :fd], tdot)
                vSTT(oV0, tdot, PW0 + 4)

                # ============ R1c: V12 ============
                ps = ppool.tile([128, 3072], F32, tag="pmix")
                for d in range(3):
                    mm(p0(d), 9, a1t[:, d, :fd])
                sb_v12 = smpool.tile([128, 3, FD], F32, tag="sb_v12")
                nc.scalar.copy(sb_v12[:, :, :fd], p0v(slice(0, 3)))

                # ============ R2b: M02 ============
                ps = ppool.tile([128, 3072], F32, tag="pmix")
                ps3 = ps.rearrange("p (a b) -> p a b", b=st2)
                for dd in range(9):
                    mm(p2(dd), 11, a2t[:, dd, :fd])
                t9b = tmp9()
                nc.vector.tensor_tensor(t9b[:, :, :fd], ps3[:, 0:9, :fd],
                                        bc2(sb_s[:, 2, :fd], 9), op=MULT)
                vSTT(oV2, t9b[:, :, :fd], PW2 + 1)
                t03 = tmp()
                diag_red(t9b[:, :, :fd], t03)
                vSTT(oV0, t03, PW0 + 3)

                # ============ R3: M12 ============
                ps = ppool.tile([128, 3072], F32, tag="pmix")
                ps3 = ps.rearrange("p (a b) -> p a b", b=st2)
                for dd in range(9):
                    mm(p2(dd), 12, a2t[:, dd, :fd])
                # b1_2: out1[f] += pv * sum_d V12_d * M12[3d+f]
                t9L = tmp9f()
                outL = t9L.rearrange("p (f z d) -> p d f z",
                                     f=3, d=3)[:, :, :, :fd]
                in0L = bcA(sb_v12[:, :, :fd])                      # (d, f*, z)
                in1L = ps3[:, 0:9, :st2].rearrange(
                    "p (d f) z -> p d f z", d=3)[:, :, :, :fd]
                nc.vector.tensor_tensor(outL, in0L, in1L, op=MULT)
                t3r = tpool.tile([128, 3, FD], F32, tag="t3", name="t3r")
                nc.vector.tensor_reduce(
                    t3r[:, :, :fd],
                    t9L.rearrange("p (f z d) -> p f z d",
                                  f=3, d=3)[:, :, :fd, :],
                    axis=AXX, op=ADD)
                vSTT(oV1, t3r[:, :, :fd], PW1 + 2)
                # b1_3: out1[f] += pv * sum_d V12_d * M12[3f+d]
                t9M = tmp9f()
                outM = t9M.rearrange("p (f z d) -> p d f z",
                                     f=3, d=3)[:, :, :, :fd]
                in1M = ps3[:, 0:9, :st2].rearrange(
                    "p (f d) z -> p d f z", f=3)[:, :, :, :fd]
                nc.vector.tensor_tensor(outM, in0L, in1M, op=MULT)
                t3s = tpool.tile([128, 3, FD], F32, tag="t3", name="t3s")
                nc.vector.tensor_reduce(
                    t3s[:, :, :fd],
                    t9M.rearrange("p (f z d) -> p f z d",
                                  f=3, d=3)[:, :, :fd, :],
                    axis=AXX, op=ADD)
                vSTT(oV1, t3s[:, :, :fd], PW1 + 3)
                # b1_4: out1_d += pv * V12_d * tr(M12)
                ttr12 = tmp()
                diag_red(ps3[:, 0:9, :fd], ttr12)
                t3t = tpool.tile([128, 3, FD], F32, tag="t3", name="t3t")
                nc.vector.tensor_tensor(t3t[:, :, :fd], sb_v12[:, :, :fd],
                                        bc2(ttr12, 3), op=MULT)
                vSTT(oV1, t3t[:, :, :fd], PW1 + 4)

                # ============ R4: A22 (psum) + B22 (sbuf) ============
                ps = ppool.tile([128, 3072], F32, tag="pmix")
                ps3 = ps.rearrange("p (a b) -> p a b", b=st2)
                for dd in range(9):
                    mm(p2(dd), 13, a2t[:, dd, :fd])
                sb_b = abpool.tile([128, 9, FD], BF16, tag="sb_b")
                sb_a = abpool.tile([128, 9, FD], BF16, tag="sb_a")
                for dd0 in range(0, 9, 2):
                    ndd = min(2, 9 - dd0)
                    stg = spool.tile([128, 512], F32, tag="stage", name="stg")
                    for i in range(ndd):
                        mm(stg[:, i * st2:i * st2 + fd], 14,
                           a2t[:, dd0 + i, :fd])
                    nc.scalar.copy(
                        sb_b[:, dd0:dd0 + ndd, :fd],
                        stg.rearrange("p (a b) -> p a b",
                                      b=st2)[:, 0:ndd, :fd])
                nc.scalar.copy(sb_a[:, 0:5, :fd], ps3[:, 0:5, :fd])
                nc.scalar.copy(sb_a[:, 5:9, :fd], ps3[:, 5:9, :fd])

                ta = tmp()
                diag_red(ps3[:, 0:9, :fd], ta)
                tb = tmp()
                diag_red(sb_b[:, :, :fd], tb)
                # b0_5: tr(A)*tr(B)
                t5 = tmp()
                nc.gpsimd.tensor_tensor(t5, ta, tb, op=MULT)
                vSTT(oV0, t5, PW0 + 5)
                # b0_6: sum(A.B)
                t9c = tmp9f()
                c_out = t9c.rearrange("p (z dd) -> p dd z", dd=9)[:, :, :fd]
                nc.vector.tensor_tensor(c_out, sb_a[:, :, :fd],
                                        sb_b[:, :, :fd], op=MULT)
                r06 = tmp()
                nc.vector.tensor_reduce(
                    r06, t9c.rearrange("p (z dd) -> p z dd", dd=9)[:, :fd, :],
                    axis=AXX, op=ADD)
                vSTT(oV0, r06, PW0 + 6)
                # b0_7: sum(A.B^T)
                t9d = tmp9f()
                d_out = t9d.rearrange("p (z d e) -> p d e z",
                                      d=3, e=3)[:, :, :, :fd]
                in0d = sb_a[:, 0:9, :FD].rearrange(
                    "p (d e) z -> p d e z", d=3)[:, :, :, :fd]
                in1d = sb_b.rearrange("p (e d) z -> p d e z",
                                      e=3)[:, :, :, :fd]
                nc.vector.tensor_tensor(d_out, in0d, in1d, op=MULT)
                r07 = tmp()
                nc.vector.tensor_reduce(
                    r07, t9d.rearrange("p (z de) -> p z de",
                                       de=9)[:, :fd, :],
                    axis=AXX, op=ADD)
                vSTT(oV0, r07, PW0 + 7)
                # b2_3: tr(A)*B product on GpSimd, accumulate on V
                t9e = tmp9()
                nc.gpsimd.tensor_tensor(t9e[:, :, :fd], sb_b[:, :, :fd],
                                        bc2(ta, 9), op=MULT)
                vSTT(oV2, t9e[:, :, :fd], PW2 + 3)
                # b2_8: A*tr(B)
                t9f = tmp9()
                nc.vector.tensor_tensor(t9f[:, :, :fd], sb_a[:, :, :fd],
                                        bc2(tb, 9), op=MULT)
                vSTT(oV2, t9f[:, :, :fd], PW2 + 8)
                # matrix products, V side: b2_4 (A^T B), b2_6 (A B)
                g4 = []
                for d in range(3):
                    t9g = tmp9()
                    nc.vector.tensor_tensor(
                        nat9(t9g), bcA(sb_a[:, 3 * d:3 * d + 3, :fd]),
                        bcB(sb_b[:, 3 * d:3 * d + 3, :fd]), op=MULT)
                    g4.append(t9g)
                nc.vector.tensor_tensor(g4[0][:, :, :fd], g4[0][:, :, :fd],
                                        g4[1][:, :, :fd], op=ADD)
                nc.vector.tensor_tensor(g4[0][:, :, :fd], g4[0][:, :, :fd],
                                        g4[2][:, :, :fd], op=ADD)
                vSTT(oV2, g4[0][:, :, :fd], PW2 + 4)
                g6 = []
                for e in range(3):
                    t9h = tmp9()
                    nc.vector.tensor_tensor(
                        nat9(t9h), bcA(sb_a[:, e:9:3, :fd]),
                        bcB(sb_b[:, 3 * e:3 * e + 3, :fd]), op=MULT)
                    g6.append(t9h)
                nc.vector.tensor_tensor(g6[0][:, :, :fd], g6[0][:, :, :fd],
                                        g6[1][:, :, :fd], op=ADD)
                nc.vector.tensor_tensor(g6[0][:, :, :fd], g6[0][:, :, :fd],
                                        g6[2][:, :, :fd], op=ADD)
                vSTT(oV2, g6[0][:, :, :fd], PW2 + 6)
                # matrix products, G side: b2_5 (A^T B^T), b2_7 (A B^T)
                g5 = []
                for d in range(3):
                    t9i = tmp9()
                    nc.gpsimd.tensor_tensor(
                        nat9(t9i), bcA(sb_a[:, 3 * d:3 * d + 3, :fd]),
                        bcB(sb_b[:, d:9:3, :fd]), op=MULT)
                    g5.append(t9i)
                nc.vector.tensor_tensor(g5[0][:, :, :fd], g5[0][:, :, :fd],
                                        g5[1][:, :, :fd], op=ADD)
                nc.vector.tensor_tensor(g5[0][:, :, :fd], g5[0][:, :, :fd],
                                        g5[2][:, :, :fd], op=ADD)
                vSTT(oV2, g5[0][:, :, :fd], PW2 + 5)
                g7 = []
                for e in range(3):
                    t9j = tmp9()
                    nc.gpsimd.tensor_tensor(
                        nat9(t9j), bcA(sb_a[:, e:9:3, :fd]),
                        bcB(sb_b[:, e:9:3, :fd]), op=MULT)
                    g7.append(t9j)
                nc.vector.tensor_tensor(g7[0][:, :, :fd], g7[0][:, :, :fd],
                                        g7[1][:, :, :fd], op=ADD)
                nc.vector.tensor_tensor(g7[0][:, :, :fd], g7[0][:, :, :fd],
                                        g7[2][:, :, :fd], op=ADD)
                vSTT(oV2, g7[0][:, :, :fd], PW2 + 7)

                nc.sync.dma_start(outd[:, :, :, zsl], outV[:, :, :fd])

    return nc


def _prep_inputs(inputs):
    """Host prep: shard over nodes, pack layouts per core."""
    a0 = np.ascontiguousarray(inputs["a0"], dtype=np.float32)
    a1 = np.ascontiguousarray(inputs["a1"], dtype=np.float32)
    a2 = np.ascontiguousarray(inputs["a2"], dtype=np.float32)
    mixes = {k: np.asarray(inputs[k], dtype=np.float32) for k in
             ("mix_0", "mix_1", "mix_2", "mix_00", "mix_01", "mix_02",
              "mix_11", "mix_12", "mix_22")}
    pw = [np.asarray(inputs["pw%d" % i], dtype=np.float32) for i in range(3)]

    wts = np.zeros((128, 15, 128), np.float32)
    for k, (name, row) in enumerate(MIX_KEYS):
        M = mixes[name][row]
        Mt = M.T
        wts[0:64, k, 0:64] = Mt
        wts[64:128, k, 64:128] = Mt

    pwv = np.zeros((128, N_PW), np.float32)
    cols = np.concatenate([pw[0], pw[1], pw[2]], axis=0)
    pwv[0:64, :] = cols.T
    pwv[64:128, :] = cols.T

    in_maps = []
    for core in range(N_CORES):
        s = slice(core * NPC, (core + 1) * NPC)

        def pack(a, ncomp):
            v = a[s].reshape(2, HALF, C, ncomp)
            return np.ascontiguousarray(v.transpose(0, 2, 3, 1))

        in_maps.append({
            "a0d": pack(a0.reshape(N_NODES, C, 1), 1).astype(ml_dtypes.bfloat16),
            "a1d": pack(a1, 3).astype(ml_dtypes.bfloat16),
            "a2d": pack(a2.reshape(N_NODES, C, 9), 9).astype(ml_dtypes.bfloat16),
            "wts": wts.astype(ml_dtypes.bfloat16),
            "pwv": pwv,
        })
    return in_maps


_CACHED_NC = None


def _get_nc():
    global _CACHED_NC
    if _CACHED_NC is None:
        _CACHED_NC = build_nc()
        _split_waits(_CACHED_NC)
    return _CACHED_NC


def run(inputs, trace=False):
    nc = _get_nc()
    in_maps = _prep_inputs(inputs)
    res = run_bass_kernel_spmd(nc, in_maps, core_ids=list(range(N_CORES)),
                               trace=trace)
    outs = []
    for core in range(N_CORES):
        o = res.results[core]["outd"]            # (2, 64, 13, HALF)
        o = o.transpose(0, 3, 1, 2).reshape(NPC, C, 13)
        outs.append(o)
    full = np.concatenate(outs, axis=0)
    out0 = np.ascontiguousarray(full[:, :, 0])
    out1 = np.ascontiguousarray(full[:, :, 1:4])
    out2 = np.ascontiguousarray(full[:, :, 4:13]).reshape(N_NODES, C, 3, 3)
    return (out0, out1, out2), res


def kernel(**inputs):
    outs, _ = run(inputs, trace=False)
    return outs


# revision 19
# speedup vs baseline: 1.2568x; 1.0234x over previous
"""Trainium2 Bass kernel for nn_CartesianEquivariantBlock (gnn_message_passing).

Data-parallel over nodes: 30000 nodes sharded 8 ways (3750/core). All
channel-mix (CxC) and path-weight (PxC) params are tiny and replicated.

Per-core device layout: partitions = (h, c) with h = node-half (2) and
c = channel (64); free dim = node index within half (1875), processed in
column tiles of FD. Channel mixing runs on TensorE (bf16, block-diagonal
128x128 stationary built on host). Bilinear spatial contractions run as
fused multi-plane tensor_tensor products (broadcast APs) on VectorE /
GpSimd with weight+accumulate via scalar_tensor_tensor / broadcast
multiplies; ScalarE does pw-scaled PSUM->SBUF copies. Output is a single
(2,64,13,1875) tensor per core, reassembled on host.
"""
import sys

import numpy as np

sys.path.insert(0, "/opt/trn_rl_repo")

import concourse.bass as bass  # noqa: E402
import ml_dtypes  # noqa: E402
import concourse.mybir as mybir  # noqa: E402
from concourse.tile import TileContext  # noqa: E402
from concourse.bass_utils import run_bass_kernel_spmd  # noqa: E402

N_CORES = 8
N_NODES = 30000
NPC = N_NODES // N_CORES      # 3750 nodes per core
C = 64
HALF = NPC // 2               # 1875 columns per node-half
FD = 256                      # node columns per tile (per half)
F32 = mybir.dt.float32
BF16 = mybir.dt.bfloat16
MULT = mybir.AluOpType.mult
ADD = mybir.AluOpType.add
BYPASS = mybir.AluOpType.bypass
AXX = mybir.AxisListType.X

# Mix order in the packed weight tensor (host side prep below):
# rank0: S0, S00a, S00b, S01, S02 -> a0-mixes (w 0..4)
# rank1: V1, V01, V11a, V11b, V12 -> a1-mixes (w 5..9)
# rank2: M2, M02, M12, A22, B22   -> a2-mixes (w 10..14)
MIX_KEYS = [
    ("mix_0", 0), ("mix_00", 0), ("mix_00", 1), ("mix_01", 0), ("mix_02", 0),
    ("mix_1", 0), ("mix_01", 1), ("mix_11", 0), ("mix_11", 1), ("mix_12", 0),
    ("mix_2", 0), ("mix_02", 1), ("mix_12", 1), ("mix_22", 0), ("mix_22", 1),
]
# pw vector columns: b0 paths 0..7 -> 0..7, b1 0..4 -> 8..12, b2 0..8 -> 13..21
PW0, PW1, PW2 = 0, 8, 13
N_PW = 22


def _plane_stride(fd):
    for s in (64, 128, 256, 512):
        if fd <= s:
            return s
    raise ValueError(fd)


def _split_waits(nc, cap=1):
    """walrus ISA structs accept very few sync waits per instruction; move
    excess waits onto same-engine no-ops inserted just before (engine
    streams are in-order, so waiting earlier is equivalent)."""
    cnt = [0]

    def process(block):
        il = getattr(block, "instructions", None)
        if il is not None:
            i = 0
            while i < len(il):
                ins = il[i]
                si = ins.sync_info
                waits = list(si.on_wait) if (si and si.on_wait) else []
                if len(waits) > cap:
                    keep = waits[-cap:]
                    extra = waits[:-cap]
                    pos = i
                    for j in range(0, len(extra), cap):
                        chunk = extra[j:j + cap]
                        cnt[0] += 1
                        nop = mybir.InstNoOp(name="waitnop%d" % cnt[0],
                                             ins=[], outs=[])
                        nop.engine = ins.engine
                        nop.sync_info = mybir.SyncInfo(on_wait=chunk,
                                                       on_update=[])
                        il.insert(pos, nop)
                        pos += 1
                        i += 1
                    ins.sync_info = mybir.SyncInfo(
                        on_wait=keep,
                        on_update=list(si.on_update) if si.on_update else [])
                i += 1
        for sb in getattr(block, "blocks", []) or []:
            process(sb)

    for b in nc.m.functions[0].blocks:
        process(b)


def build_nc():
    nc = bass.Bass()
    a0d = nc.dram_tensor("a0d", [2, C, 1, HALF], BF16, kind="ExternalInput")
    a1d = nc.dram_tensor("a1d", [2, C, 3, HALF], BF16, kind="ExternalInput")
    a2d = nc.dram_tensor("a2d", [2, C, 9, HALF], BF16, kind="ExternalInput")
    wtsd = nc.dram_tensor("wts", [128, 15, 128], BF16, kind="ExternalInput")
    pwvd = nc.dram_tensor("pwv", [128, N_PW], F32, kind="ExternalInput")
    outd = nc.dram_tensor("outd", [2, C, 13, HALF], F32, kind="ExternalOutput")

    with TileContext(nc) as tc:
        with (
            tc.tile_pool(name="const", bufs=1) as cpool,
            tc.tile_pool(name="ain", bufs=3) as apool,
            tc.tile_pool(name="acc", bufs=3) as accpool,
            tc.tile_pool(name="osm", bufs=2) as smpool,
            tc.tile_pool(name="oab", bufs=1) as abpool,
            tc.tile_pool(name="tmp", bufs=6) as tpool,
            tc.tile_pool(name="t9", bufs=8) as t9pool,
            tc.tile_pool(name="pmix", bufs=1, space="PSUM") as ppool,
            tc.tile_pool(name="pstage", bufs=2, space="PSUM") as spool,
        ):
            w_sb = cpool.tile([128, 15, 128], BF16)
            nc.sync.dma_start(w_sb[:], wtsd[:])
            pw_sb = cpool.tile([128, N_PW], F32)
            nc.sync.dma_start(pw_sb[:], pwvd[:])

            def pv(k):
                return pw_sb[:, k:k + 1]

            def W(k):
                return w_sb[:, k, :]

            n_tiles = (HALF + FD - 1) // FD
            for t in range(n_tiles):
                z0 = t * FD
                fd = min(FD, HALF - z0)
                st2 = _plane_stride(fd)
                zsl = slice(z0, z0 + fd)

                a0t = apool.tile([128, 1, FD], BF16, tag="a0t")
                a1t = apool.tile([128, 3, FD], BF16, tag="a1t")
                a2t = apool.tile([128, 9, FD], BF16, tag="a2t")
                nc.sync.dma_start(a0t[:, :, :fd], a0d[:, :, :, zsl])
                nc.sync.dma_start(a1t[:, :, :fd], a1d[:, :, :, zsl])
                nc.sync.dma_start(a2t[:, :, :fd], a2d[:, :, :, zsl])

                outV = accpool.tile([128, 13, FD], F32, tag="outV")

                oV0 = outV[:, 0, :fd]
                oV1 = outV[:, 1:4, :fd]
                oV2 = outV[:, 4:13, :fd]

                def mm(psum_ap, w_idx, rhs_ap):
                    nc.tensor.matmul(psum_ap, W(w_idx), rhs_ap,
                                     start=True, stop=True)

                def vSTT(acc, x, k):
                    # acc += pv[k] * x
                    nc.vector.scalar_tensor_tensor(acc, x, pv(k), acc,
                                                   MULT, ADD)

                def tmp():
                    return tpool.tile([128, FD], F32, tag="tmp",
                                      name="tmp")[:, :fd]

                def tmp9():
                    return t9pool.tile([128, 9, FD], BF16, tag="tmp9",
                                       name="tmp9")

                def tmp9f():
                    return t9pool.tile([128, 9 * FD], F32, tag="tmp9",
                                       name="tmp9f")

                def bc2(ap2, n):
                    # [128, z] -> [128, n, z] broadcast
                    return ap2.rearrange("p (o z) -> p o z", o=1).broadcast_to(
                        (128, n, ap2.shape[-1]))

                def bc_pv3(k, n, z):
                    return pv(k).rearrange("p (a b) -> p a b", a=1).broadcast_to(
                        (128, n, z))

                def bcA(ap3):
                    # [128, a, z] -> [128, a, 3, z]  (broadcast new mid dim)
                    s = ap3.shape
                    return ap3.rearrange("p a (o z) -> p a o z", o=1).broadcast_to(
                        (128, s[1], 3, s[2]))

                def bcB(ap3):
                    # [128, b, z] -> [128, 3, b, z]
                    s = ap3.shape
                    return ap3.rearrange("p b (o z) -> p o b z", o=1).broadcast_to(
                        (128, 3, s[1], s[2]))

                def nat9(t9t):
                    # [128, 9, FD] natural (x, y, z) 4D view
                    return t9t.rearrange("p (d e) z -> p d e z", d=3)[:, :, :, :fd]

                def diag_red(ap3v, out2):
                    # reduce planes {0,4,8} of [128, 9, z] view -> [128, z]
                    dv = ap3v[:, 0:9:4, :].rearrange("p d z -> p z d")
                    nc.vector.tensor_reduce(out2, dv, axis=AXX, op=ADD)

                # ============ R0: rank-0 mixes ============
                ps = ppool.tile([128, 3072], F32, tag="pmix")

                def p0(m):
                    return ps[:, m * 512:m * 512 + fd]

                def p0v(sl):
                    return ps.rearrange("p (a b) -> p a b", b=512)[:, sl, :fd]

                for m in range(5):
                    mm(p0(m), m, a0t[:, 0, :fd])
                nc.scalar.mul(oV0, p0(0), pv(PW0 + 0))
                sb_s = smpool.tile([128, 3, FD], F32, tag="sb_s")
                nc.scalar.copy(sb_s[:, :, :fd], p0v(slice(2, 5)))
                t1 = tmp()
                nc.vector.tensor_tensor(t1, p0(1), sb_s[:, 0, :fd], op=MULT)
                vSTT(oV0, t1, PW0 + 2)

                # ============ R1a: V1, V01 ============
                ps = ppool.tile([128, 3072], F32, tag="pmix")
                for d in range(3):
                    mm(p0(d), 5, a1t[:, d, :fd])
                    mm(p0(3 + d), 6, a1t[:, d, :fd])
                nc.scalar.mul(oV1, p0v(slice(0, 3)), pv(PW1 + 0))
                t3 = tpool.tile([128, 3, FD], F32, tag="t3", name="t3")
                nc.vector.tensor_tensor(t3[:, :, :fd], p0v(slice(3, 6)),
                                        bc2(sb_s[:, 1, :fd], 3), op=MULT)
                vSTT(oV1, t3[:, :, :fd], PW1 + 1)

                # ============ R2a: M2 ============
                ps = ppool.tile([128, 3072], F32, tag="pmix")
                ps3 = ps.rearrange("p (a b) -> p a b", b=st2)

                def p2(dd):
                    return ps[:, dd * st2:dd * st2 + fd]

                for dd in range(9):
                    mm(p2(dd), 10, a2t[:, dd, :fd])
                nc.scalar.mul(oV2, ps3[:, 0:9, :fd], pv(PW2 + 0))
                ttr = tmp()
                diag_red(ps3[:, 0:9, :fd], ttr)
                vSTT(oV0, ttr, PW0 + 1)

                # ============ R1b: V11a, V11b ============
                ps = ppool.tile([128, 3072], F32, tag="pmix")
                for d in range(3):
                    mm(p0(d), 7, a1t[:, d, :fd])
                    mm(p0(3 + d), 8, a1t[:, d, :fd])
                sb_v11b = smpool.tile([128, 3, FD], F32, tag="sb_v11b")
                nc.scalar.copy(sb_v11b[:, :, :fd], p0v(slice(3, 6)))
                t9a = tmp9()
                nc.vector.tensor_tensor(
                    nat9(t9a), bcA(p0v(slice(0, 3))),
                    bcB(sb_v11b[:, :, :fd]), op=MULT)
                vSTT(oV2, t9a[:, :, :fd], PW2 + 2)
                tdot = tmp()
                diag_red(t9a[:, :, :fd], tdot)
                vSTT(oV0, tdot, PW0 + 4)

                # ============ R1c: V12 ============
                ps = ppool.tile([128, 3072], F32, tag="pmix")
                for d in range(3):
                    mm(p0(d), 9, a1t[:, d, :fd])
                sb_v12 = smpool.tile([128, 3, FD], F32, tag="sb_v12")
                nc.scalar.copy(sb_v12[:, :, :fd], p0v(slice(0, 3)))

                # ============ R2b: M02 ============
                ps = ppool.tile([128, 3072], F32, tag="pmix")
                ps3 = ps.rearrange("p (a b) -> p a b", b=st2)
                for dd in range(9):
                    mm(p2(dd), 11, a2t[:, dd, :fd])
                t9b = tmp9()
                nc.vector.tensor_tensor(t9b[:, :, :fd], ps3[:, 0:9, :fd],
                                        bc2(sb_s[:, 2, :fd], 9), op=MULT)
                vSTT(oV2, t9b[:, :, :fd], PW2 + 1)
                t03 = tmp()
                diag_red(t9b[:, :, :fd], t03)
                vSTT(oV0, t03, PW0 + 3)

                # ============ R3: M12 ============
                ps = ppool.tile([128, 3072], F32, tag="pmix")
                ps3 = ps.rearrange("p (a b) -> p a b", b=st2)
                for dd in range(9):
                    mm(p2(dd), 12, a2t[:, dd, :fd])
                # b1_2: out1[f] += pv * sum_d V12_d * M12[3d+f]
                t9L = tmp9f()
                outL = t9L.rearrange("p (f z d) -> p d f z",
                                     f=3, d=3)[:, :, :, :fd]
                in0L = bcA(sb_v12[:, :, :fd])                      # (d, f*, z)
                in1L = ps3[:, 0:9, :st2].rearrange(
                    "p (d f) z -> p d f z", d=3)[:, :, :, :fd]
                nc.vector.tensor_tensor(outL, in0L, in1L, op=MULT)
                t3r = tpool.tile([128, 3, FD], F32, tag="t3", name="t3r")
                nc.vector.tensor_reduce(
                    t3r[:, :, :fd],
                    t9L.rearrange("p (f z d) -> p f z d",
                                  f=3, d=3)[:, :, :fd, :],
                    axis=AXX, op=ADD)
                vSTT(oV1, t3r[:, :, :fd], PW1 + 2)
                # b1_3: out1[f] += pv * sum_d V12_d * M12[3f+d]
                t9M = tmp9f()
                outM = t9M.rearrange("p (f z d) -> p d f z",
                                     f=3, d=3)[:, :, :, :fd]
                in1M = ps3[:, 0:9, :st2].rearrange(
                    "p (f d) z -> p d f z", f=3)[:, :, :, :fd]
                nc.vector.tensor_tensor(outM, in0L, in1M, op=MULT)
                t3s = tpool.tile([128, 3, FD], F32, tag="t3", name="t3s")
                nc.vector.tensor_reduce(
                    t3s[:, :, :fd],
                    t9M.rearrange("p (f z d) -> p f z d",
                                  f=3, d=3)[:, :, :fd, :],
                    axis=AXX, op=ADD)
                vSTT(oV1, t3s[:, :, :fd], PW1 + 3)
                # b1_4: out1_d += pv * V12_d * tr(M12)
                ttr12 = tmp()
                diag_red(ps3[:, 0:9, :fd], ttr12)
                t3t = tpool.tile([128, 3, FD], F32, tag="t3", name="t3t")
                nc.vector.tensor_tensor(t3t[:, :, :fd], sb_v12[:, :, :fd],
                                        bc2(ttr12, 3), op=MULT)
                vSTT(oV1, t3t[:, :, :fd], PW1 + 4)

                # ============ R4: A22 (psum) + B22 (sbuf) ============
                ps = ppool.tile([128, 3072], F32, tag="pmix")
                ps3 = ps.rearrange("p (a b) -> p a b", b=st2)
                for dd in range(9):
                    mm(p2(dd), 13, a2t[:, dd, :fd])
                sb_b = abpool.tile([128, 9, FD], BF16, tag="sb_b")
                sb_a = abpool.tile([128, 9, FD], BF16, tag="sb_a")
                for dd0 in range(0, 9, 2):
                    ndd = min(2, 9 - dd0)
                    stg = spool.tile([128, 512], F32, tag="stage", name="stg")
                    for i in range(ndd):
                        mm(stg[:, i * st2:i * st2 + fd], 14,
                           a2t[:, dd0 + i, :fd])
                    nc.scalar.copy(
                        sb_b[:, dd0:dd0 + ndd, :fd],
                        stg.rearrange("p (a b) -> p a b",
                                      b=st2)[:, 0:ndd, :fd])
                nc.scalar.copy(sb_a[:, 0:5, :fd], ps3[:, 0:5, :fd])
                nc.scalar.copy(sb_a[:, 5:9, :fd], ps3[:, 5:9, :fd])

                ta = tmp()
                diag_red(ps3[:, 0:9, :fd], ta)
                tb = tmp()
                diag_red(sb_b[:, :, :fd], tb)
                # b0_5: tr(A)*tr(B)
                t5 = tmp()
                nc.gpsimd.tensor_tensor(t5, ta, tb, op=MULT)
                vSTT(oV0, t5, PW0 + 5)
                # b0_6: sum(A.B)
                t9c = tmp9f()
                c_out = t9c.rearrange("p (z dd) -> p dd z", dd=9)[:, :, :fd]
                nc.vector.tensor_tensor(c_out, sb_a[:, :, :fd],
                                        sb_b[:, :, :fd], op=MULT)
                r06 = tmp()
                nc.vector.tensor_reduce(
                    r06, t9c.rearrange("p (z dd) -> p z dd", dd=9)[:, :fd, :],
                    axis=AXX, op=ADD)
                vSTT(oV0, r06, PW0 + 6)
                # b0_7: sum(A.B^T)
                t9d = tmp9f()
                d_out = t9d.rearrange("p (z d e) -> p d e z",
                                      d=3, e=3)[:, :, :, :fd]
                in0d = sb_a[:, 0:9, :FD].rearrange(
                    "p (d e) z -> p d e z", d=3)[:, :, :, :fd]
                in1d = sb_b.rearrange("p (e d) z -> p d e z",
                                      e=3)[:, :, :, :fd]
                nc.vector.tensor_tensor(d_out, in0d, in1d, op=MULT)
                r07 = tmp()
                nc.vector.tensor_reduce(
                    r07, t9d.rearrange("p (z de) -> p z de",
                                       de=9)[:, :fd, :],
                    axis=AXX, op=ADD)
                vSTT(oV0, r07, PW0 + 7)
                # b2_3: tr(A)*B product on GpSimd, accumulate on V
                t9e = tmp9()
                nc.gpsimd.tensor_tensor(t9e[:, :, :fd], sb_b[:, :, :fd],
                                        bc2(ta, 9), op=MULT)
                vSTT(oV2, t9e[:, :, :fd], PW2 + 3)
                # b2_8: A*tr(B)
                t9f = tmp9()
                nc.vector.tensor_tensor(t9f[:, :, :fd], sb_a[:, :, :fd],
                                        bc2(tb, 9), op=MULT)
                vSTT(oV2, t9f[:, :, :fd], PW2 + 8)
                # matrix products, V side: b2_4 (A^T B), b2_6 (A B)
                g4 = []
                for d in range(3):
                    t9g = tmp9()
                    nc.vector.tensor_tensor(
                        nat9(t9g), bcA(sb_a[:, 3 * d:3 * d + 3, :fd]),
                        bcB(sb_b[:, 3 * d:3 * d + 3, :fd]), op=MULT)
                    g4.append(t9g)
                nc.vector.tensor_tensor(g4[0][:, :, :fd], g4[0][:, :, :fd],
                                        g4[1][:, :, :fd], op=ADD)
                nc.vector.tensor_tensor(g4[0][:, :, :fd], g4[0][:, :, :fd],
                                        g4[2][:, :, :fd], op=ADD)
                vSTT(oV2, g4[0][:, :, :fd], PW2 + 4)
                g6 = []
                for e in range(3):
                    t9h = tmp9()
                    nc.vector.tensor_tensor(
                        nat9(t9h), bcA(sb_a[:, e:9:3, :fd]),
                        bcB(sb_b[:, 3 * e:3 * e + 3, :fd]), op=MULT)
                    g6.append(t9h)
                nc.vector.tensor_tensor(g6[0][:, :, :fd], g6[0][:, :, :fd],
                                        g6[1][:, :, :fd], op=ADD)
                nc.vector.tensor_tensor(g6[0][:, :, :fd], g6[0][:, :, :fd],
                                        g6[2][:, :, :fd], op=ADD)
                vSTT(oV2, g6[0][:, :, :fd], PW2 + 6)
                # matrix products, G side: b2_5 (A^T B^T), b2_7 (A B^T)
                g5 = []
                for d in range(3):
                    t9i = tmp9()
                    nc.gpsimd.tensor_tensor(
                        nat9(t9i), bcA(sb_a[:, 3 * d:3 * d + 3, :fd]),
                        bcB(sb_b[:, d:9:3, :fd]), op=MULT)
                    g5.append(t9i)
                nc.vector.tensor_tensor(g5[0][:, :, :fd], g5[0][:, :, :fd],
                                        g5[1][:, :, :fd], op=ADD)
                nc.vector.tensor_tensor(g5[0][:, :, :fd], g5[0][:, :, :fd],
                                        g5[2][:, :, :fd], op=ADD)
                vSTT(oV2, g5[0][:, :, :fd], PW2 + 5)
                g7 = []
                for e in range(3):
                    t9j = tmp9()
                    nc.gpsimd.tensor_tensor(
                        nat9(t9j), bcA(sb_a[:, e:9:3, :fd]),
                        bcB(sb_b[:, e:9:3, :fd]), op=MULT)
                    g7.append(t9j)
                nc.vector.tensor_tensor(g7[0][:, :, :fd], g7[0][:, :, :fd],
                                        g7[1][:, :, :fd], op=ADD)
                nc.vector.tensor_tensor(g7[0][:, :, :fd], g7[0][:, :, :fd],
                                        g7[2][:, :, :fd], op=ADD)
                vSTT(oV2, g7[0][:, :, :fd], PW2 + 7)

                nc.sync.dma_start(outd[:, :, :, zsl], outV[:, :, :fd])

    return nc


def _prep_inputs(inputs):
    """Host prep: shard over nodes, pack layouts per core."""
    a0 = np.ascontiguousarray(inputs["a0"], dtype=np.float32)
    a1 = np.ascontiguousarray(inputs["a1"], dtype=np.float32)
    a2 = np.ascontiguousarray(inputs["a2"], dtype=np.float32)
    mixes = {k: np.asarray(inputs[k], dtype=np.float32) for k in
             ("mix_0", "mix_1", "mix_2", "mix_00", "mix_01", "mix_02",
              "mix_11", "mix_12", "mix_22")}
    pw = [np.asarray(inputs["pw%d" % i], dtype=np.float32) for i in range(3)]

    wts = np.zeros((128, 15, 128), np.float32)
    for k, (name, row) in enumerate(MIX_KEYS):
        M = mixes[name][row]
        Mt = M.T
        wts[0:64, k, 0:64] = Mt
        wts[64:128, k, 64:128] = Mt

    pwv = np.zeros((128, N_PW), np.float32)
    cols = np.concatenate([pw[0], pw[1], pw[2]], axis=0)
    pwv[0:64, :] = cols.T
    pwv[64:128, :] = cols.T

    in_maps = []
    for core in range(N_CORES):
        s = slice(core * NPC, (core + 1) * NPC)

        def pack(a, ncomp):
            v = a[s].reshape(2, HALF, C, ncomp)
            return np.ascontiguousarray(v.transpose(0, 2, 3, 1))

        in_maps.append({
            "a0d": pack(a0.reshape(N_NODES, C, 1), 1).astype(ml_dtypes.bfloat16),
            "a1d": pack(a1, 3).astype(ml_dtypes.bfloat16),
            "a2d": pack(a2.reshape(N_NODES, C, 9), 9).astype(ml_dtypes.bfloat16),
            "wts": wts.astype(ml_dtypes.bfloat16),
            "pwv": pwv,
        })
    return in_maps


_CACHED_NC = None


def _get_nc():
    global _CACHED_NC
    if _CACHED_NC is None:
        _CACHED_NC = build_nc()
        _split_waits(_CACHED_NC)
    return _CACHED_NC


def run(inputs, trace=False):
    nc = _get_nc()
    in_maps = _prep_inputs(inputs)
    res = run_bass_kernel_spmd(nc, in_maps, core_ids=list(range(N_CORES)),
                               trace=trace)
    outs = []
    for core in range(N_CORES):
        o = res.results[core]["outd"]            # (2, 64, 13, HALF)
        o = o.transpose(0, 3, 1, 2).reshape(NPC, C, 13)
        outs.append(o)
    full = np.concatenate(outs, axis=0)
    out0 = np.ascontiguousarray(full[:, :, 0])
    out1 = np.ascontiguousarray(full[:, :, 1:4])
    out2 = np.ascontiguousarray(full[:, :, 4:13]).reshape(N_NODES, C, 3, 3)
    return (out0, out1, out2), res


def kernel(**inputs):
    outs, _ = run(inputs, trace=False)
    return outs
